# revision 7
# baseline (speedup 1.0000x reference)
"""Causal multi-head attention with RoPE on 8 Trainium2 NeuronCores (v4).

Sharding: core c -> batch b = c // 2, head-group g = c % 2 (8 heads each).
Each core computes q/k/v projections for its 512 output dims, RoPE, causal
attention for its 8 heads, and a partial O-projection. Host sums the two
partial outputs per batch and transposes back.

Dataflow (all bf16 except where noted):
  - x resident in SBUF: bf16 copy for the V projection, fp8e4 copy for the
    Q/K projections (fp8 DoubleRow matmuls contract two 128-row k-subtiles
    per instruction at ~1.5x bf16 rate; wq/wk uploaded fp8 pre-scaled by 64
    so their ~1e-3 values stay in fp8 normal range, compensated in the
    softmax exp scale).
  - Scores/PV/O-proj in bf16; exp on the scalar engine straight from PSUM;
    causal masking one affine_select over both heads; denominator from a
    ones-column appended to V.
  - q/k stored per-chunk ([128, 512] tiles) so a score matmul only waits on
    RoPE of the j/i ranges it reads, not the whole pair.
  - Attention inner loop software-pipelined: scores(jt+1) issue before
    PV(jt) so the PE never waits on exp; next head-pair's projections and
    the O-projection are interleaved as hooks between j-tiles.
"""

import os
import numpy as np

import concourse.bass as bass
import concourse.tile as tile
from concourse import bacc, mybir
from concourse.bass_utils import run_bass_kernel_spmd

F32 = mybir.dt.float32
BF16 = mybir.dt.bfloat16
F8 = mybir.dt.float8e4
DR = mybir.MatmulPerfMode.DoubleRow
MULT = mybir.AluOpType.mult
IS_GE = mybir.AluOpType.is_ge
EXP = mybir.ActivationFunctionType.Exp

WSCALE = 64.0     # wq/wk pre-scaled by 64 on host so fp8e4 stays in normals
ESCALE = 0.125 / (WSCALE * WSCALE)

P = 128          # partitions
S = 2048         # sequence length
D = 1024         # model dim
DK = 64          # head dim
HPC = 8          # heads per core
NPAIR = 4        # head pairs per core
KT = 8           # 128-row k-tiles of the contraction dim (D)
CH = 512         # i-chunk width
NCH = S // CH    # 4 i-chunks
NJT = S // P     # 16 j-tiles

_CACHED_NC = None
LAST_RESULTS = None


def build_nc():
    nc = bacc.Bacc("TRN2", target_bir_lowering=False, debug=False)

    xT = nc.dram_tensor("xT", [D, S], BF16, kind="ExternalInput").ap()
    x8 = nc.dram_tensor("x8", [D, S], F8, kind="ExternalInput").ap()
    wq = nc.dram_tensor("wq", [D, 512], F8, kind="ExternalInput").ap()
    wk = nc.dram_tensor("wk", [D, 512], F8, kind="ExternalInput").ap()
    wv = nc.dram_tensor("wv", [D, 512], BF16, kind="ExternalInput").ap()
    wo = nc.dram_tensor("wo", [512, D], BF16, kind="ExternalInput").ap()
    cosn = nc.dram_tensor("cosn", [P, S], BF16, kind="ExternalInput").ap()
    sins = nc.dram_tensor("sins", [P, S], BF16, kind="ExternalInput").ap()
    psw = nc.dram_tensor("psw", [P, P], BF16, kind="ExternalInput").ap()
    out = nc.dram_tensor("out", [D, S], BF16, kind="ExternalOutput").ap()

    xT3 = xT.rearrange("(kt p) s -> p kt s", p=P)
    x83 = x8.rearrange("(kt p) s -> p kt s", p=P)
    wq3 = wq.rearrange("(kt p) o -> p kt o", p=P)
    wk3 = wk.rearrange("(kt p) o -> p kt o", p=P)
    wv3 = wv.rearrange("(kt p) o -> p kt o", p=P)
    wo3 = wo.rearrange("(pt p) o -> p pt o", p=P)

    with tile.TileContext(nc) as tc:
        with tc.tile_pool(name="persist", bufs=1) as persist:
            x8_sb = persist.tile([P, KT, S], F8, tag="x8")
            cos_sb = persist.tile([P, S], BF16, tag="cos")
            sin_sb = persist.tile([P, S], BF16, tag="sin")
            psw_sb = persist.tile([P, P], BF16, tag="psw")

            v_sb = [persist.tile([P, HPC * 65], BF16, name=f"v{jt}", tag=f"v{jt}") for jt in range(NJT)]
            wo_sb = persist.tile([P, NPAIR, D], BF16, tag="wo")
            ones8 = persist.tile([P, HPC], BF16, tag="ones8")
            nc.vector.memset(ones8[:], 1.0)
            # touch Exp early so the ~2.7us ACT table load overlaps DMAs
            nc.scalar.activation(ones8[0:1, :], ones8[0:1, :], EXP, scale=0.0)
            att_sb = [persist.tile([P, S], BF16, name=f"att{p}", tag=f"att{p}") for p in range(NPAIR)]

            # ---- Phase 1: V projection, all heads at once (N=512) ----
            with (
                tc.tile_pool(name="p1w", bufs=1) as p1w,
                tc.tile_pool(name="pp1", bufs=1, space="PSUM") as pp1,
            ):
                wv_sb = p1w.tile([P, KT, 512], BF16, tag="wv")
                x_sb = p1w.tile([P, KT, S], BF16, tag="x")
                nc.sync.dma_start(wv_sb[:, 0:2, :], wv3[:, 0:2, :])
                nc.sync.dma_start(wv_sb[:, 2:KT, :], wv3[:, 2:KT, :])
                # x loads in column waves so V-proj can start on wave 0
                for w in range(NCH):
                    ssl = slice(w * CH, (w + 1) * CH)
                    for kt in range(KT):
                        nc.sync.dma_start(x_sb[:, kt, ssl], xT3[:, kt, ssl])
                    nc.sync.dma_start(x8_sb[:, :, ssl], x83[:, :, ssl])
                    nc.sync.dma_start(cos_sb[:, ssl], cosn[:, ssl])
                    nc.sync.dma_start(sin_sb[:, ssl], sins[:, ssl])
                nc.sync.dma_start(psw_sb[:], psw)
                for st in range(NJT):
                    ps = pp1.tile([P, 512], F32, tag="p1ps", bufs=2)
                    for kt in range(KT):
                        nc.tensor.matmul(
                            ps[:],
                            x_sb[:, kt, st * P:(st + 1) * P],
                            wv_sb[:, kt, :],
                            start=(kt == 0),
                            stop=(kt == KT - 1),
                        )
                    v3 = v_sb[st][:].rearrange("p (h e) -> p h e", e=65)
                    nc.vector.tensor_copy(
                        out=v3[:, :, 0:DK],
                        in_=ps[:].rearrange("p (h d) -> p h d", d=DK),
                    )
                    nc.vector.tensor_copy(
                        out=v3[:, :, DK:65], in_=ones8[:, :, None])

            # ---- Phases 2+3 per head pair ----
            pair_ctx = [
                tc.tile_pool(name="work", bufs=2),
                tc.tile_pool(name="tmp", bufs=2),
                tc.tile_pool(name="expp", bufs=3),
                tc.tile_pool(name="pp23", bufs=1, space="PSUM"),
            ]
            work, tmp, expp, pp = [c.__enter__() for c in pair_ctx]

            def p2_prefetch(pair):
                st = {}
                # per-chunk q/k tiles: a score matmul depends only on the
                # chunk it reads, not on the whole pair's RoPE completing
                st["q"] = [work.tile([P, CH], BF16, tag=f"qp{c}", name=f"q{pair}_{c}")
                           for c in range(NCH)]
                st["k"] = [work.tile([P, CH], BF16, tag=f"kp{c}", name=f"k{pair}_{c}")
                           for c in range(NCH)]
                st["wq"] = work.tile([P, KT, P], F8, tag="wqp", name=f"wq{pair}")
                st["wk"] = work.tile([P, KT, P], F8, tag="wkp", name=f"wk{pair}")
                osl = slice(pair * P, (pair + 1) * P)
                nc.sync.dma_start(st["wq"][:], wq3[:, :, osl])
                nc.sync.dma_start(st["wk"][:], wk3[:, :, osl])
                return st

            def p2_mm(st, c, which):
                # fp8 DoubleRow projection burst: 4 matmuls, each contracting
                # two 128-row k-subtiles; then raw copy for the RoPE stage
                ssl = slice(c * CH, (c + 1) * CH)
                w_t = st["wq"] if which == "q" else st["wk"]
                if which == "q":
                    st["ps2"] = pp.tile([P, 2, CH], F32, tag="ps2", bufs=1, name="ps2t")
                ps2 = st["ps2"]
                for kq in range(KT // 2):
                    nc.tensor.matmul(
                        ps2[:, 0, :], w_t[:, 2 * kq:2 * kq + 2, :],
                        x8_sb[:, 2 * kq:2 * kq + 2, ssl],
                        start=(kq == 0), stop=(kq == KT // 2 - 1),
                        perf_mode=DR)
                raw = tmp.tile([P, CH], BF16, tag="raw")
                nc.vector.tensor_copy(out=raw[:], in_=ps2[:, 0, :])
                st["raw"] = raw

            def p2_rope(st, c, which):
                # RoPE: dst = raw * cos + swap(raw) * sin_signed
                ssl = slice(c * CH, (c + 1) * CH)
                dst = st["q"][c] if which == "q" else st["k"][c]
                ps2, raw = st["ps2"], st["raw"]
                nc.tensor.matmul(
                    ps2[:, 1, :], psw_sb[:], raw[:],
                    start=True, stop=True)
                tsin = tmp.tile([P, CH], BF16, tag="tsin")
                nc.vector.tensor_tensor(tsin[:], ps2[:, 1, :], sin_sb[:, ssl], MULT)
                tcos = tmp.tile([P, CH], BF16, tag="tcos")
                nc.vector.tensor_tensor(tcos[:], raw[:], cos_sb[:, ssl], MULT)
                nc.vector.tensor_add(out=dst[:], in0=tcos[:], in1=tsin[:])

            def p2_hooks(st, c):
                return [
                    lambda: p2_mm(st, c, "q"),
                    lambda: p2_rope(st, c, "q"),
                    lambda: p2_mm(st, c, "k"),
                    lambda: p2_rope(st, c, "k"),
                ]

            def p3_chunk(pair, st, c, hooks):
                # hooks: {jt_index: [fn]} emitted between jt iterations to
                # interleave projection bursts / O-proj into the PE queue
                h0c, h1c = 65 * (2 * pair), 65 * (2 * pair + 1)
                ssl = slice(c * CH, (c + 1) * CH)
                q_t = st["q"][c]
                # one bank holds both heads' PV accumulation (col-tiled, the
                # two matmuls co-execute); a second bank holds denominators:
                # row 64/96 = head A/B, seeded by a rank-1 sum-of-scores
                # matmul for the off-diagonal j range (exp(s)-1 ~= s there),
                # then accumulated exactly by ones-matmuls over diag tiles
                psAB = pp.tile([P, CH], F32, tag="pvA", bufs=1, name="psAB")
                psDen = pp.tile([97, CH], F32, tag="pvB", bufs=1, name="psDen")
                njt = 4 * c + 4
                if c > 0:
                    kred = tmp.tile([P, 1], F32, tag="kred")
                    nc.vector.tensor_reduce(
                        out=kred[:], in_=st["k"][c - 1][:],
                        axis=mybir.AxisListType.X, op=mybir.AluOpType.add)
                    if c == 1:
                        ksum = kred
                    else:
                        ksum = tmp.tile([P, 1], F32, tag="ksacc")
                        nc.vector.tensor_add(
                            out=ksum[:], in0=kred[:], in1=st["ksum"][:])
                    st["ksum"] = ksum
                    ks8 = tmp.tile([P, 1], BF16, tag="ks8")
                    nc.vector.tensor_scalar(
                        ks8[:], ksum[:], ESCALE, None, MULT)
                    nc.tensor.matmul(
                        psDen[64:65, :], ks8[0:DK, :], q_t[0:DK, :],
                        start=True, stop=False, tile_position=(0, 64),
                        skip_group_check=True)
                    nc.tensor.matmul(
                        psDen[96:97, :], ks8[DK:P, :], q_t[DK:P, :],
                        start=True, stop=False, tile_position=(64, 96),
                        skip_group_check=True)

                def pv(jt, ex, start_, w_):
                    # start=True clears the whole PSUM bank on HW, so only
                    # the FIRST matmul touching each bank per chunk sets it;
                    # later writers to untouched regions overwrite (per-elem
                    # has_written=0 after the clear)
                    first, last = (jt == 0), (jt == njt - 1)
                    nc.tensor.matmul(
                        psAB[0:DK, start_:], v_sb[jt][:, h0c:h0c + DK],
                        ex[:, 0, start_:], start=first, stop=last,
                        tile_position=(0, 0), skip_group_check=True)
                    nc.tensor.matmul(
                        psAB[DK:P, start_:], v_sb[jt][:, h1c:h1c + DK],
                        ex[:, 1, start_:], start=first, stop=last,
                        tile_position=(0, 64), skip_group_check=True)
                    if jt >= 4 * c:
                        dfirst = (c == 0 and jt == 0)
                        nc.tensor.matmul(
                            psDen[64:65, start_:], ones8[:, 0:1],
                            ex[:, 0, start_:], start=dfirst, stop=last,
                            tile_position=(0, 64), skip_group_check=True)
                        nc.tensor.matmul(
                            psDen[96:97, start_:], ones8[:, 0:1],
                            ex[:, 1, start_:], start=dfirst, stop=last,
                            tile_position=(0, 96), skip_group_check=True)

                prev = None
                for jt in range(njt):
                    for fn in hooks.get(jt, ()):
                        fn()
                    start = max(0, (jt - 4 * c) * P)
                    w = CH - start
                    jsl = slice((jt % 4) * P, (jt % 4 + 1) * P)
                    k_t = st["k"][jt // 4]
                    q_t = st["q"][c]
                    sc = pp.tile([P, 2, CH], F32, tag="sc", bufs=2)
                    nc.tensor.matmul(
                        sc[:, 0, start:], k_t[0:DK, jsl], q_t[0:DK, start:],
                        start=True, stop=True, tile_position=(0, 0))
                    nc.tensor.matmul(
                        sc[:, 1, start:], k_t[DK:P, jsl], q_t[DK:P, start:],
                        start=True, stop=True, tile_position=(DK, 0))
                    ex = expp.tile([P, 2, CH], BF16, tag="exp")
                    nc.scalar.activation(
                        ex[:, :, start:], sc[:, :, start:], EXP, scale=ESCALE)
                    if jt >= 4 * c:
                        nc.gpsimd.affine_select(
                            out=ex[:, :, start:], in_=ex[:, :, start:],
                            compare_op=IS_GE, fill=0.0,
                            base=c * CH + start - jt * P,
                            channel_multiplier=-1,
                            pattern=[[0, 2], [1, w]])
                    if prev is not None:
                        pv(*prev)
                    prev = (jt, ex, start, w)
                pv(*prev)
                # both heads' denominators (psDen rows 64 and 96) processed
                # in single 2-partition strided ops
                # head A multiply is partition-aligned at base 0; head B at
                # base 64 uses a full-width broadcast of its reciprocal so
                # every operand of each DVE op shares the same partition base
                for hoff, drow in ((0, 64), (DK, 96)):
                    d0 = tmp.tile([1, CH], F32, tag="d0")
                    nc.vector.tensor_scalar(
                        d0[:], psDen[drow:drow + 1, :], float(c * CH), None,
                        mybir.AluOpType.add)
                    rcp = tmp.tile([1, CH], F32, tag="rcp")
                    nc.vector.reciprocal_approx_fast(out=rcp[:], in_=d0[:])
                    nch = DK if hoff == 0 else P
                    bc = tmp.tile([P, CH], F32, tag=f"bc{hoff}", name=f"bc{hoff}")
                    nc.gpsimd.partition_broadcast(
                        bc[0:nch, :], rcp[:], channels=nch)
                    nc.vector.tensor_tensor(
                        att_sb[pair][hoff:hoff + DK, ssl],
                        psAB[hoff:hoff + DK, :], bc[hoff:hoff + DK, :], MULT)

            pso_box = {}

            def p4_group(ot, c, scalar_copy=False):
                ssl = slice(c * CH, (c + 1) * CH)
                pso = pso_box["pso"][:, ot % 2, :]
                for p_ in range(NPAIR):
                    nc.tensor.matmul(
                        pso,
                        wo_sb[:, p_, ot * P:(ot + 1) * P],
                        att_sb[p_][:, ssl],
                        start=(p_ == 0), stop=(p_ == NPAIR - 1))
                ob = tmp.tile([P, CH], BF16, tag="ob")
                if scalar_copy:
                    nc.scalar.copy(out=ob[:], in_=pso)
                else:
                    nc.vector.tensor_copy(out=ob[:], in_=pso)
                nc.sync.dma_start(out[ot * P:(ot + 1) * P, ssl], ob[:])

            # prologue: projections for pair 0 chunk 0 only; the rest of
            # pair 0's P2 is hosted inside its own attention chunks
            st_cur = p2_prefetch(0)
            st_next = p2_prefetch(1)
            for fn in p2_hooks(st_cur, 0):
                fn()
            for pair in range(NPAIR):
                if pair == NPAIR - 1:
                    nc.sync.dma_start(wo_sb[:], wo3)
                    pso_box["pso"] = pp.tile([P, 2, CH], F32, tag="ps2", bufs=1, name="psot")
                for c in range(NCH):
                    hooks = {}
                    njt = 4 * c + 4

                    def add_unit(fns, lo, hi):
                        # spread 4 sub-hooks over jt slots [lo, hi)
                        span = max(1, hi - lo)
                        for idx, fn in enumerate(fns):
                            hooks.setdefault(lo + idx * span // 4, []).append(fn)

                    if pair == 0 and c < NCH - 1:
                        add_unit(p2_hooks(st_cur, c + 1), 0, njt // 2)
                        add_unit(p2_hooks(st_next, c), njt // 2, njt)
                    elif pair == 0:
                        add_unit(p2_hooks(st_next, c), 0, njt)
                    elif pair < NPAIR - 1:
                        add_unit(p2_hooks(st_next, c), njt // 5, njt)
                    elif c > 0:
                        # interleave O-projection of chunk c-1 into this
                        # chunk, starting a few j-tiles in so the previous
                        # chunk's normalize (DVE) has time to land
                        npts = min(4, njt - 3)
                        for gi in range(8):
                            key = 3 + (gi % npts) * (njt - 3) // npts
                            hooks.setdefault(key, []).append(
                                lambda o=gi, cc=c - 1: p4_group(o, cc))
                    p3_chunk(pair, st_cur, c, hooks)
                st_cur = st_next
                st_next = p2_prefetch(pair + 2) if pair + 2 < NPAIR else None
            for ot in range(D // P):
                p4_group(ot, NCH - 1, scalar_copy=True)

            for c_ in reversed(pair_ctx):
                c_.__exit__(None, None, None)

    nc.compile()
    return nc


def _get_nc():
    global _CACHED_NC
    if _CACHED_NC is None:
        _CACHED_NC = build_nc()
    return _CACHED_NC


def make_in_maps(x, token_positions, Wq, Wk, Wv, Wo):
    import ml_dtypes
    bf = ml_dtypes.bfloat16
    f8 = ml_dtypes.float8_e4m3
    x = np.asarray(x, dtype=np.float32)
    pos = np.asarray(token_positions).astype(np.float64)

    freq_idx = np.arange(0, DK, 2, dtype=np.float64)
    inv_freq = 1.0 / (10000.0 ** (freq_idx / DK))
    ang = pos[:, None] * inv_freq[None, :]          # [S, DK/2]
    cos_t = np.cos(ang).astype(np.float32).T        # [DK/2, S]
    sin_t = np.sin(ang).astype(np.float32).T

    pidx = (np.arange(P) % DK) // 2
    cosn = np.ascontiguousarray(cos_t[pidx, :]).astype(bf)
    sgn = np.where(np.arange(P) % 2 == 0, -1.0, 1.0).astype(np.float32)
    sins = np.ascontiguousarray(sin_t[pidx, :] * sgn[:, None]).astype(bf)

    psw = np.zeros((P, P), dtype=np.float32)
    psw[np.arange(P), np.arange(P) ^ 1] = 1.0
    psw = psw.astype(bf)

    in_maps = []
    for core in range(8):
        b, g = core // 2, core % 2
        sl = slice(512 * g, 512 * g + 512)
        in_maps.append({
            "xT": np.ascontiguousarray(x[b].T).astype(bf),
            "x8": np.ascontiguousarray(x[b].T).astype(f8),
            "wq": (np.ascontiguousarray(np.asarray(Wq)[sl, :].T) * WSCALE).astype(f8),
            "wk": (np.ascontiguousarray(np.asarray(Wk)[sl, :].T) * WSCALE).astype(f8),
            "wv": np.ascontiguousarray(np.asarray(Wv)[sl, :].T).astype(bf),
            "wo": np.ascontiguousarray(np.asarray(Wo)[:, sl].T).astype(bf),
            "cosn": cosn,
            "sins": sins,
            "psw": psw,
        })
    return in_maps


def kernel(x, token_positions, Wq, Wk, Wv, Wo):
    global LAST_RESULTS
    nc = _get_nc()
    in_maps = make_in_maps(x, token_positions, Wq, Wk, Wv, Wo)
    res = run_bass_kernel_spmd(nc, in_maps, list(range(8)))
    LAST_RESULTS = res
    B = x.shape[0]
    outp = np.empty((B, S, D), dtype=np.float32)
    for b in range(B):
        outp[b] = (res.results[2 * b]["out"].astype(np.float32)
                   + res.results[2 * b + 1]["out"].astype(np.float32)).T
    return outp


# revision 8
# speedup vs baseline: 1.0590x; 1.0590x over previous
"""Causal multi-head attention with RoPE on 8 Trainium2 NeuronCores (v4).

Sharding: core c -> batch b = c // 2, head-group g = c % 2 (8 heads each).
Each core computes q/k/v projections for its 512 output dims, RoPE, causal
attention for its 8 heads, and a partial O-projection. Host sums the two
partial outputs per batch and transposes back.

Dataflow (all bf16 except where noted):
  - x resident in SBUF: bf16 copy for the V projection, fp8e4 copy for the
    Q/K projections (fp8 DoubleRow matmuls contract two 128-row k-subtiles
    per instruction at ~1.5x bf16 rate; wq/wk uploaded fp8 pre-scaled by 64
    so their ~1e-3 values stay in fp8 normal range, compensated in the
    softmax exp scale).
  - Scores/PV/O-proj in bf16; exp on the scalar engine straight from PSUM;
    causal masking one affine_select over both heads; denominator from a
    ones-column appended to V.
  - q/k stored per-chunk ([128, 512] tiles) so a score matmul only waits on
    RoPE of the j/i ranges it reads, not the whole pair.
  - Attention inner loop software-pipelined: scores(jt+1) issue before
    PV(jt) so the PE never waits on exp; next head-pair's projections and
    the O-projection are interleaved as hooks between j-tiles.
"""

import os
import numpy as np

import concourse.bass as bass
import concourse.tile as tile
from concourse import bacc, mybir
from concourse.bass_utils import run_bass_kernel_spmd

F32 = mybir.dt.float32
BF16 = mybir.dt.bfloat16
F8 = mybir.dt.float8e4
DR = mybir.MatmulPerfMode.DoubleRow
MULT = mybir.AluOpType.mult
IS_GE = mybir.AluOpType.is_ge
EXP = mybir.ActivationFunctionType.Exp

WSCALE = 64.0     # wq/wk pre-scaled by 64 on host so fp8e4 stays in normals
ESCALE = 0.125 / (WSCALE * WSCALE)

P = 128          # partitions
S = 2048         # sequence length
D = 1024         # model dim
DK = 64          # head dim
HPC = 8          # heads per core
NPAIR = 4        # head pairs per core
KT = 8           # 128-row k-tiles of the contraction dim (D)
CH = 512         # i-chunk width
NCH = S // CH    # 4 i-chunks
NJT = S // P     # 16 j-tiles

_CACHED_NC = None
LAST_RESULTS = None


def build_nc():
    nc = bacc.Bacc("TRN2", target_bir_lowering=False, debug=False)

    xT = nc.dram_tensor("xT", [D, S], BF16, kind="ExternalInput").ap()
    x8 = nc.dram_tensor("x8", [D, S], F8, kind="ExternalInput").ap()
    wq = nc.dram_tensor("wq", [D, 512], F8, kind="ExternalInput").ap()
    wk = nc.dram_tensor("wk", [D, 512], F8, kind="ExternalInput").ap()
    wv = nc.dram_tensor("wv", [D, 512], BF16, kind="ExternalInput").ap()
    wo = nc.dram_tensor("wo", [512, D], BF16, kind="ExternalInput").ap()
    cosn = nc.dram_tensor("cosn", [P, S], BF16, kind="ExternalInput").ap()
    sins = nc.dram_tensor("sins", [P, S], BF16, kind="ExternalInput").ap()
    psw = nc.dram_tensor("psw", [P, P], BF16, kind="ExternalInput").ap()
    out = nc.dram_tensor("out", [D, S], BF16, kind="ExternalOutput").ap()

    xT3 = xT.rearrange("(kt p) s -> p kt s", p=P)
    x83 = x8.rearrange("(kt p) s -> p kt s", p=P)
    wq3 = wq.rearrange("(kt p) o -> p kt o", p=P)
    wk3 = wk.rearrange("(kt p) o -> p kt o", p=P)
    wv3 = wv.rearrange("(kt p) o -> p kt o", p=P)
    wo3 = wo.rearrange("(pt p) o -> p pt o", p=P)

    with tile.TileContext(nc) as tc:
        with tc.tile_pool(name="persist", bufs=1) as persist:
            x8_sb = persist.tile([P, KT, S], F8, tag="x8")
            cos_sb = persist.tile([P, S], BF16, tag="cos")
            sin_sb = persist.tile([P, S], BF16, tag="sin")
            psw_sb = persist.tile([P, P], BF16, tag="psw")

            v_sb = [persist.tile([P, HPC * 65], BF16, name=f"v{jt}", tag=f"v{jt}") for jt in range(NJT)]
            wo_sb = persist.tile([P, NPAIR, D], BF16, tag="wo")
            ones8 = persist.tile([P, HPC], BF16, tag="ones8")
            nc.vector.memset(ones8[:], 1.0)
            # touch Exp early so the ~2.7us ACT table load overlaps DMAs
            nc.scalar.activation(ones8[0:1, :], ones8[0:1, :], EXP, scale=0.0)
            att_sb = [persist.tile([P, S], BF16, name=f"att{p}", tag=f"att{p}") for p in range(NPAIR)]

            # ---- Phase 1: V projection, all heads at once (N=512) ----
            with (
                tc.tile_pool(name="p1w", bufs=1) as p1w,
                tc.tile_pool(name="pp1", bufs=1, space="PSUM") as pp1,
            ):
                wv_sb = p1w.tile([P, KT, 512], BF16, tag="wv")
                x_sb = p1w.tile([P, KT, S], BF16, tag="x")
                nc.sync.dma_start(wv_sb[:, 0:2, :], wv3[:, 0:2, :])
                nc.sync.dma_start(wv_sb[:, 2:KT, :], wv3[:, 2:KT, :])
                # x loads in column waves so V-proj can start on wave 0
                for w in range(NCH):
                    ssl = slice(w * CH, (w + 1) * CH)
                    for kt in range(KT):
                        nc.sync.dma_start(x_sb[:, kt, ssl], xT3[:, kt, ssl])
                    nc.sync.dma_start(x8_sb[:, :, ssl], x83[:, :, ssl])
                    nc.sync.dma_start(cos_sb[:, ssl], cosn[:, ssl])
                    nc.sync.dma_start(sin_sb[:, ssl], sins[:, ssl])
                nc.sync.dma_start(psw_sb[:], psw)
                for st in range(NJT):
                    ps = pp1.tile([P, 512], F32, tag="p1ps", bufs=2)
                    for kt in range(KT):
                        nc.tensor.matmul(
                            ps[:],
                            x_sb[:, kt, st * P:(st + 1) * P],
                            wv_sb[:, kt, :],
                            start=(kt == 0),
                            stop=(kt == KT - 1),
                        )
                    v3 = v_sb[st][:].rearrange("p (h e) -> p h e", e=65)
                    nc.vector.tensor_copy(
                        out=v3[:, :, 0:DK],
                        in_=ps[:].rearrange("p (h d) -> p h d", d=DK),
                    )
                    nc.vector.tensor_copy(
                        out=v3[:, :, DK:65], in_=ones8[:, :, None])

            # ---- Phases 2+3 per head pair ----
            pair_ctx = [
                tc.tile_pool(name="work", bufs=2),
                tc.tile_pool(name="tmp", bufs=2),
                tc.tile_pool(name="expp", bufs=3),
                tc.tile_pool(name="pp23", bufs=1, space="PSUM"),
            ]
            work, tmp, expp, pp = [c.__enter__() for c in pair_ctx]

            def p2_prefetch(pair):
                st = {}
                # per-chunk q/k tiles: a score matmul depends only on the
                # chunk it reads, not on the whole pair's RoPE completing
                st["q"] = [work.tile([P, CH], BF16, tag=f"qp{c}", name=f"q{pair}_{c}")
                           for c in range(NCH)]
                st["k"] = [work.tile([P, CH], BF16, tag=f"kp{c}", name=f"k{pair}_{c}")
                           for c in range(NCH)]
                st["wq"] = work.tile([P, KT, P], F8, tag="wqp", name=f"wq{pair}")
                st["wk"] = work.tile([P, KT, P], F8, tag="wkp", name=f"wk{pair}")
                osl = slice(pair * P, (pair + 1) * P)
                nc.sync.dma_start(st["wq"][:], wq3[:, :, osl])
                nc.sync.dma_start(st["wk"][:], wk3[:, :, osl])
                return st

            def p2_mm(st, c, which):
                # fp8 DoubleRow projection burst: 4 matmuls, each contracting
                # two 128-row k-subtiles; then raw copy for the RoPE stage
                ssl = slice(c * CH, (c + 1) * CH)
                w_t = st["wq"] if which == "q" else st["wk"]
                if which == "q":
                    st["ps2"] = pp.tile([P, 2, CH], F32, tag="ps2", bufs=1, name="ps2t")
                ps2 = st["ps2"]
                for kq in range(KT // 2):
                    nc.tensor.matmul(
                        ps2[:, 0, :], w_t[:, 2 * kq:2 * kq + 2, :],
                        x8_sb[:, 2 * kq:2 * kq + 2, ssl],
                        start=(kq == 0), stop=(kq == KT // 2 - 1),
                        perf_mode=DR)
                raw = tmp.tile([P, CH], BF16, tag="raw")
                nc.vector.tensor_copy(out=raw[:], in_=ps2[:, 0, :])
                st["raw"] = raw

            def p2_rope(st, c, which):
                # RoPE: dst = raw * cos + swap(raw) * sin_signed
                ssl = slice(c * CH, (c + 1) * CH)
                dst = st["q"][c] if which == "q" else st["k"][c]
                ps2, raw = st["ps2"], st["raw"]
                nc.tensor.matmul(
                    ps2[:, 1, :], psw_sb[:], raw[:],
                    start=True, stop=True)
                tsin = tmp.tile([P, CH], BF16, tag="tsin")
                nc.vector.tensor_tensor(tsin[:], ps2[:, 1, :], sin_sb[:, ssl], MULT)
                tcos = tmp.tile([P, CH], BF16, tag="tcos")
                nc.vector.tensor_tensor(tcos[:], raw[:], cos_sb[:, ssl], MULT)
                nc.vector.tensor_add(out=dst[:], in0=tcos[:], in1=tsin[:])

            def p2_hooks(st, c):
                return [
                    lambda: p2_mm(st, c, "q"),
                    lambda: p2_rope(st, c, "q"),
                    lambda: p2_mm(st, c, "k"),
                    lambda: p2_rope(st, c, "k"),
                ]

            def p3_chunk(pair, st, c, hooks):
                # hooks: {jt_index: [fn]} emitted between jt iterations to
                # interleave projection bursts / O-proj into the PE queue
                h0c, h1c = 65 * (2 * pair), 65 * (2 * pair + 1)
                ssl = slice(c * CH, (c + 1) * CH)
                psA = pp.tile([65, CH], F32, tag="pvA", bufs=1)
                psB = pp.tile([65, CH], F32, tag="pvB", bufs=1)
                njt = 4 * c + 4

                def pv(jt, ex, start_, w_):
                    first, last = (jt == 0), (jt == njt - 1)
                    nc.tensor.matmul(
                        psA[:, start_:], v_sb[jt][:, h0c:h0c + 65],
                        ex[:, 0, start_:], start=first, stop=last)
                    nc.tensor.matmul(
                        psB[:, start_:], v_sb[jt][:, h1c:h1c + 65],
                        ex[:, 1, start_:], start=first, stop=last)

                prev = None
                for jt in range(njt):
                    for fn in hooks.get(jt, ()):
                        fn()
                    start = max(0, (jt - 4 * c) * P)
                    w = CH - start
                    jsl = slice((jt % 4) * P, (jt % 4 + 1) * P)
                    k_t = st["k"][jt // 4]
                    q_t = st["q"][c]
                    sc = pp.tile([P, 2, CH], F32, tag="sc", bufs=2)
                    nc.tensor.matmul(
                        sc[:, 0, start:], k_t[0:DK, jsl], q_t[0:DK, start:],
                        start=True, stop=True, tile_position=(0, 0))
                    nc.tensor.matmul(
                        sc[:, 1, start:], k_t[DK:P, jsl], q_t[DK:P, start:],
                        start=True, stop=True, tile_position=(DK, 0))
                    ex = expp.tile([P, 2, CH], BF16, tag="exp")
                    nc.scalar.activation(
                        ex[:, :, start:], sc[:, :, start:], EXP, scale=ESCALE)
                    if jt >= 4 * c:
                        nc.gpsimd.affine_select(
                            out=ex[:, :, start:], in_=ex[:, :, start:],
                            compare_op=IS_GE, fill=0.0,
                            base=c * CH + start - jt * P,
                            channel_multiplier=-1,
                            pattern=[[0, 2], [1, w]])
                    if prev is not None:
                        pv(*prev)
                    prev = (jt, ex, start, w)
                pv(*prev)
                for ps_, hoff in ((psA, 0), (psB, DK)):
                    d0 = tmp.tile([1, CH], F32, tag="d0")
                    nc.vector.tensor_copy(out=d0[:], in_=ps_[DK:DK + 1, :])
                    rcp = tmp.tile([1, CH], F32, tag="rcp")
                    nc.vector.reciprocal_approx_fast(out=rcp[:], in_=d0[:])
                    bc = tmp.tile([DK, CH], F32, tag="bc")
                    nc.gpsimd.partition_broadcast(bc[:], rcp[:], channels=DK)
                    nc.vector.tensor_tensor(
                        att_sb[pair][hoff:hoff + DK, ssl],
                        ps_[0:DK, :], bc[:], MULT)

            pso_box = {}

            def p4_group(ot, c, scalar_copy=False):
                ssl = slice(c * CH, (c + 1) * CH)
                pso = pso_box["pso"][:, ot % 2, :]
                for p_ in range(NPAIR):
                    nc.tensor.matmul(
                        pso,
                        wo_sb[:, p_, ot * P:(ot + 1) * P],
                        att_sb[p_][:, ssl],
                        start=(p_ == 0), stop=(p_ == NPAIR - 1))
                ob = tmp.tile([P, CH], BF16, tag="ob")
                if scalar_copy:
                    nc.scalar.copy(out=ob[:], in_=pso)
                else:
                    nc.vector.tensor_copy(out=ob[:], in_=pso)
                nc.sync.dma_start(out[ot * P:(ot + 1) * P, ssl], ob[:])

            # prologue: projections for pair 0 chunk 0 only; the rest of
            # pair 0's P2 is hosted inside its own attention chunks
            st_cur = p2_prefetch(0)
            st_next = p2_prefetch(1)
            for fn in p2_hooks(st_cur, 0):
                fn()
            for pair in range(NPAIR):
                if pair == NPAIR - 1:
                    nc.sync.dma_start(wo_sb[:], wo3)
                    pso_box["pso"] = pp.tile([P, 2, CH], F32, tag="ps2", bufs=1, name="psot")
                for c in range(NCH):
                    hooks = {}
                    njt = 4 * c + 4

                    def add_unit(fns, lo, hi):
                        # spread 4 sub-hooks over jt slots [lo, hi)
                        span = max(1, hi - lo)
                        for idx, fn in enumerate(fns):
                            hooks.setdefault(lo + idx * span // 4, []).append(fn)

                    if pair == 0 and c < NCH - 1:
                        add_unit(p2_hooks(st_cur, c + 1), 0, njt // 2)
                        add_unit(p2_hooks(st_next, c), njt // 2, njt)
                    elif pair == 0:
                        add_unit(p2_hooks(st_next, c), 0, njt)
                    elif pair < NPAIR - 1:
                        add_unit(p2_hooks(st_next, c), njt // 5, njt)
                    elif c > 0:
                        # interleave O-projection of chunk c-1 into this
                        # chunk, starting a few j-tiles in so the previous
                        # chunk's normalize (DVE) has time to land
                        npts = min(4, njt - 3)
                        for gi in range(8):
                            key = 3 + (gi % npts) * (njt - 3) // npts
                            hooks.setdefault(key, []).append(
                                lambda o=gi, cc=c - 1: p4_group(o, cc, scalar_copy=(gi % 2 == 0)))
                    p3_chunk(pair, st_cur, c, hooks)
                st_cur = st_next
                st_next = p2_prefetch(pair + 2) if pair + 2 < NPAIR else None
            for ot in range(D // P):
                p4_group(ot, NCH - 1, scalar_copy=(ot % 2 == 0))

            for c_ in reversed(pair_ctx):
                c_.__exit__(None, None, None)

    nc.compile()
    return nc


def _get_nc():
    global _CACHED_NC
    if _CACHED_NC is None:
        _CACHED_NC = build_nc()
    return _CACHED_NC


def make_in_maps(x, token_positions, Wq, Wk, Wv, Wo):
    import ml_dtypes
    bf = ml_dtypes.bfloat16
    f8 = ml_dtypes.float8_e4m3
    x = np.asarray(x, dtype=np.float32)
    pos = np.asarray(token_positions).astype(np.float64)

    freq_idx = np.arange(0, DK, 2, dtype=np.float64)
    inv_freq = 1.0 / (10000.0 ** (freq_idx / DK))
    ang = pos[:, None] * inv_freq[None, :]          # [S, DK/2]
    cos_t = np.cos(ang).astype(np.float32).T        # [DK/2, S]
    sin_t = np.sin(ang).astype(np.float32).T

    pidx = (np.arange(P) % DK) // 2
    cosn = np.ascontiguousarray(cos_t[pidx, :]).astype(bf)
    sgn = np.where(np.arange(P) % 2 == 0, -1.0, 1.0).astype(np.float32)
    sins = np.ascontiguousarray(sin_t[pidx, :] * sgn[:, None]).astype(bf)

    psw = np.zeros((P, P), dtype=np.float32)
    psw[np.arange(P), np.arange(P) ^ 1] = 1.0
    psw = psw.astype(bf)

    in_maps = []
    for core in range(8):
        b, g = core // 2, core % 2
        sl = slice(512 * g, 512 * g + 512)
        in_maps.append({
            "xT": np.ascontiguousarray(x[b].T).astype(bf),
            "x8": np.ascontiguousarray(x[b].T).astype(f8),
            "wq": (np.ascontiguousarray(np.asarray(Wq)[sl, :].T) * WSCALE).astype(f8),
            "wk": (np.ascontiguousarray(np.asarray(Wk)[sl, :].T) * WSCALE).astype(f8),
            "wv": np.ascontiguousarray(np.asarray(Wv)[sl, :].T).astype(bf),
            "wo": np.ascontiguousarray(np.asarray(Wo)[:, sl].T).astype(bf),
            "cosn": cosn,
            "sins": sins,
            "psw": psw,
        })
    return in_maps


def kernel(x, token_positions, Wq, Wk, Wv, Wo):
    global LAST_RESULTS
    nc = _get_nc()
    in_maps = make_in_maps(x, token_positions, Wq, Wk, Wv, Wo)
    res = run_bass_kernel_spmd(nc, in_maps, list(range(8)))
    LAST_RESULTS = res
    B = x.shape[0]
    outp = np.empty((B, S, D), dtype=np.float32)
    for b in range(B):
        outp[b] = (res.results[2 * b]["out"].astype(np.float32)
                   + res.results[2 * b + 1]["out"].astype(np.float32)).T
    return outp


# revision 9
# speedup vs baseline: 1.0829x; 1.0226x over previous
"""Causal multi-head attention with RoPE on 8 Trainium2 NeuronCores (v4).

Sharding: core c -> batch b = c // 2, head-group g = c % 2 (8 heads each).
Each core computes q/k/v projections for its 512 output dims, RoPE, causal
attention for its 8 heads, and a partial O-projection. Host sums the two
partial outputs per batch and transposes back.

Dataflow (all bf16 except where noted):
  - x resident in SBUF: bf16 copy for the V projection, fp8e4 copy for the
    Q/K projections (fp8 DoubleRow matmuls contract two 128-row k-subtiles
    per instruction at ~1.5x bf16 rate; wq/wk uploaded fp8 pre-scaled by 64
    so their ~1e-3 values stay in fp8 normal range, compensated in the
    softmax exp scale).
  - Scores/PV/O-proj in bf16; exp on the scalar engine straight from PSUM;
    causal masking one affine_select over both heads; denominator from a
    ones-column appended to V.
  - q/k stored per-chunk ([128, 512] tiles) so a score matmul only waits on
    RoPE of the j/i ranges it reads, not the whole pair.
  - Attention inner loop software-pipelined: scores(jt+1) issue before
    PV(jt) so the PE never waits on exp; next head-pair's projections and
    the O-projection are interleaved as hooks between j-tiles.
"""

import os
import numpy as np

import concourse.bass as bass
import concourse.tile as tile
from concourse import bacc, mybir
from concourse.bass_utils import run_bass_kernel_spmd

F32 = mybir.dt.float32
BF16 = mybir.dt.bfloat16
F8 = mybir.dt.float8e4
DR = mybir.MatmulPerfMode.DoubleRow
MULT = mybir.AluOpType.mult
IS_GE = mybir.AluOpType.is_ge
EXP = mybir.ActivationFunctionType.Exp

WSCALE = 64.0     # wq/wk pre-scaled by 64 on host so fp8e4 stays in normals
ESCALE = 0.125 / (WSCALE * WSCALE)

P = 128          # partitions
S = 2048         # sequence length
D = 1024         # model dim
DK = 64          # head dim
HPC = 8          # heads per core
NPAIR = 4        # head pairs per core
KT = 8           # 128-row k-tiles of the contraction dim (D)
CH = 512         # i-chunk width
NCH = S // CH    # 4 i-chunks
NJT = S // P     # 16 j-tiles

_CACHED_NC = None
LAST_RESULTS = None


def build_nc():
    nc = bacc.Bacc("TRN2", target_bir_lowering=False, debug=False)

    xT = nc.dram_tensor("xT", [D, S], BF16, kind="ExternalInput").ap()
    x8 = nc.dram_tensor("x8", [D, S], F8, kind="ExternalInput").ap()
    wq = nc.dram_tensor("wq", [D, 512], F8, kind="ExternalInput").ap()
    wk = nc.dram_tensor("wk", [D, 512], F8, kind="ExternalInput").ap()
    wv = nc.dram_tensor("wv", [D, 512], BF16, kind="ExternalInput").ap()
    wo = nc.dram_tensor("wo", [512, D], BF16, kind="ExternalInput").ap()
    cosn = nc.dram_tensor("cosn", [P, S], BF16, kind="ExternalInput").ap()
    sins = nc.dram_tensor("sins", [P, S], BF16, kind="ExternalInput").ap()
    psw = nc.dram_tensor("psw", [P, P], BF16, kind="ExternalInput").ap()
    out = nc.dram_tensor("out", [D, S], BF16, kind="ExternalOutput").ap()

    xT3 = xT.rearrange("(kt p) s -> p kt s", p=P)
    x83 = x8.rearrange("(kt p) s -> p kt s", p=P)
    wq3 = wq.rearrange("(kt p) o -> p kt o", p=P)
    wk3 = wk.rearrange("(kt p) o -> p kt o", p=P)
    wv3 = wv.rearrange("(kt p) o -> p kt o", p=P)
    wo3 = wo.rearrange("(pt p) o -> p pt o", p=P)

    with tile.TileContext(nc) as tc:
        with tc.tile_pool(name="persist", bufs=1) as persist:
            x8_sb = persist.tile([P, KT, S], F8, tag="x8")
            cos_sb = persist.tile([P, S], BF16, tag="cos")
            sin_sb = persist.tile([P, S], BF16, tag="sin")
            psw_sb = persist.tile([P, P], BF16, tag="psw")

            v_sb = [persist.tile([P, HPC * 65], BF16, name=f"v{jt}", tag=f"v{jt}") for jt in range(NJT)]
            wo_sb = persist.tile([P, NPAIR, D], BF16, tag="wo")
            ones8 = persist.tile([P, HPC], BF16, tag="ones8")
            nc.vector.memset(ones8[:], 1.0)
            # touch Exp early so the ~2.7us ACT table load overlaps DMAs
            nc.scalar.activation(ones8[0:1, :], ones8[0:1, :], EXP, scale=0.0)
            att_sb = [persist.tile([P, S], BF16, name=f"att{p}", tag=f"att{p}") for p in range(NPAIR)]

            # ---- Phase 1: V projection, all heads at once (N=512) ----
            with (
                tc.tile_pool(name="p1w", bufs=1) as p1w,
                tc.tile_pool(name="pp1", bufs=1, space="PSUM") as pp1,
            ):
                wv_sb = p1w.tile([P, KT, 512], BF16, tag="wv")
                x_sb = p1w.tile([P, KT, S], BF16, tag="x")
                nc.sync.dma_start(wv_sb[:, 0:2, :], wv3[:, 0:2, :])
                nc.sync.dma_start(wv_sb[:, 2:KT, :], wv3[:, 2:KT, :])
                # x loads in column waves so V-proj can start on wave 0
                for w in range(NCH):
                    ssl = slice(w * CH, (w + 1) * CH)
                    for kt in range(KT):
                        nc.sync.dma_start(x_sb[:, kt, ssl], xT3[:, kt, ssl])
                    nc.sync.dma_start(x8_sb[:, :, ssl], x83[:, :, ssl])
                    nc.sync.dma_start(cos_sb[:, ssl], cosn[:, ssl])
                    nc.sync.dma_start(sin_sb[:, ssl], sins[:, ssl])
                nc.sync.dma_start(psw_sb[:], psw)
                for st in range(NJT):
                    ps = pp1.tile([P, 512], F32, tag="p1ps", bufs=2)
                    for kt in range(KT):
                        nc.tensor.matmul(
                            ps[:],
                            x_sb[:, kt, st * P:(st + 1) * P],
                            wv_sb[:, kt, :],
                            start=(kt == 0),
                            stop=(kt == KT - 1),
                        )
                    v3 = v_sb[st][:].rearrange("p (h e) -> p h e", e=65)
                    nc.vector.tensor_copy(
                        out=v3[:, :, 0:DK],
                        in_=ps[:].rearrange("p (h d) -> p h d", d=DK),
                    )
                    nc.vector.tensor_copy(
                        out=v3[:, :, DK:65], in_=ones8[:, :, None])

            # ---- Phases 2+3 per head pair ----
            pair_ctx = [
                tc.tile_pool(name="work", bufs=2),
                tc.tile_pool(name="tmp", bufs=2),
                tc.tile_pool(name="expp", bufs=3),
                tc.tile_pool(name="pp23", bufs=1, space="PSUM"),
            ]
            work, tmp, expp, pp = [c.__enter__() for c in pair_ctx]

            def p2_prefetch(pair):
                st = {}
                # per-chunk q/k tiles: a score matmul depends only on the
                # chunk it reads, not on the whole pair's RoPE completing
                st["q"] = [work.tile([P, CH], BF16, tag=f"qp{c}", name=f"q{pair}_{c}")
                           for c in range(NCH)]
                st["k"] = [work.tile([P, CH], BF16, tag=f"kp{c}", name=f"k{pair}_{c}")
                           for c in range(NCH)]
                st["wq"] = work.tile([P, KT, P], F8, tag="wqp", name=f"wq{pair}")
                st["wk"] = work.tile([P, KT, P], F8, tag="wkp", name=f"wk{pair}")
                osl = slice(pair * P, (pair + 1) * P)
                nc.sync.dma_start(st["wq"][:], wq3[:, :, osl])
                nc.sync.dma_start(st["wk"][:], wk3[:, :, osl])
                return st

            def p2_mm(st, c, which):
                # fp8 DoubleRow projection burst: 4 matmuls, each contracting
                # two 128-row k-subtiles; then raw copy for the RoPE stage
                ssl = slice(c * CH, (c + 1) * CH)
                w_t = st["wq"] if which == "q" else st["wk"]
                if which == "q":
                    st["ps2"] = pp.tile([P, 2, CH], F32, tag="ps2", bufs=1, name="ps2t")
                ps2 = st["ps2"]
                for kq in range(KT // 2):
                    nc.tensor.matmul(
                        ps2[:, 0, :], w_t[:, 2 * kq:2 * kq + 2, :],
                        x8_sb[:, 2 * kq:2 * kq + 2, ssl],
                        start=(kq == 0), stop=(kq == KT // 2 - 1),
                        perf_mode=DR)
                raw = tmp.tile([P, CH], BF16, tag="raw")
                nc.vector.tensor_copy(out=raw[:], in_=ps2[:, 0, :])
                st["raw"] = raw

            def p2_rope(st, c, which):
                # RoPE: dst = raw * cos + swap(raw) * sin_signed
                ssl = slice(c * CH, (c + 1) * CH)
                dst = st["q"][c] if which == "q" else st["k"][c]
                ps2, raw = st["ps2"], st["raw"]
                nc.tensor.matmul(
                    ps2[:, 1, :], psw_sb[:], raw[:],
                    start=True, stop=True)
                tsin = tmp.tile([P, CH], BF16, tag="tsin")
                nc.vector.tensor_tensor(tsin[:], ps2[:, 1, :], sin_sb[:, ssl], MULT)
                tcos = tmp.tile([P, CH], BF16, tag="tcos")
                nc.vector.tensor_tensor(tcos[:], raw[:], cos_sb[:, ssl], MULT)
                nc.vector.tensor_add(out=dst[:], in0=tcos[:], in1=tsin[:])

            def p2_hooks(st, c):
                return [
                    lambda: p2_mm(st, c, "q"),
                    lambda: p2_rope(st, c, "q"),
                    lambda: p2_mm(st, c, "k"),
                    lambda: p2_rope(st, c, "k"),
                ]

            def p3_chunk(pair, st, c, hooks):
                # hooks: {jt_index: [fn]} emitted between jt iterations to
                # interleave projection bursts / O-proj into the PE queue
                h0c, h1c = 65 * (2 * pair), 65 * (2 * pair + 1)
                ssl = slice(c * CH, (c + 1) * CH)
                psA = pp.tile([65, CH], F32, tag="pvA", bufs=1)
                psB = pp.tile([65, CH], F32, tag="pvB", bufs=1)
                njt = 4 * c + 4

                def pv(jt, ex, start_, w_):
                    first, last = (jt == 0), (jt == njt - 1)
                    nc.tensor.matmul(
                        psA[:, start_:], v_sb[jt][:, h0c:h0c + 65],
                        ex[:, 0, start_:], start=first, stop=last)
                    nc.tensor.matmul(
                        psB[:, start_:], v_sb[jt][:, h1c:h1c + 65],
                        ex[:, 1, start_:], start=first, stop=last)

                prev = None
                for jt in range(njt):
                    for fn in hooks.get(jt, ()):
                        fn()
                    start = max(0, (jt - 4 * c) * P)
                    w = CH - start
                    jsl = slice((jt % 4) * P, (jt % 4 + 1) * P)
                    k_t = st["k"][jt // 4]
                    q_t = st["q"][c]
                    sc = pp.tile([P, 2, CH], F32, tag="sc", bufs=2)
                    nc.tensor.matmul(
                        sc[:, 0, start:], k_t[0:DK, jsl], q_t[0:DK, start:],
                        start=True, stop=True, tile_position=(0, 0))
                    nc.tensor.matmul(
                        sc[:, 1, start:], k_t[DK:P, jsl], q_t[DK:P, start:],
                        start=True, stop=True, tile_position=(DK, 0))
                    ex = expp.tile([P, 2, CH], BF16, tag="exp")
                    nc.scalar.activation(
                        ex[:, :, start:], sc[:, :, start:], EXP, scale=ESCALE)
                    if jt >= 4 * c:
                        nc.gpsimd.affine_select(
                            out=ex[:, :, start:], in_=ex[:, :, start:],
                            compare_op=IS_GE, fill=0.0,
                            base=c * CH + start - jt * P,
                            channel_multiplier=-1,
                            pattern=[[0, 2], [1, w]])
                    if prev is not None:
                        pv(*prev)
                    prev = (jt, ex, start, w)
                pv(*prev)
                for ps_, hoff in ((psA, 0), (psB, DK)):
                    d0 = tmp.tile([1, CH], F32, tag="d0")
                    nc.vector.tensor_copy(out=d0[:], in_=ps_[DK:DK + 1, :])
                    rcp = tmp.tile([1, CH], F32, tag="rcp")
                    nc.vector.reciprocal_approx_fast(out=rcp[:], in_=d0[:])
                    bc = tmp.tile([DK, CH], F32, tag="bc")
                    nc.gpsimd.partition_broadcast(bc[:], rcp[:], channels=DK)
                    nc.vector.tensor_tensor(
                        att_sb[pair][hoff:hoff + DK, ssl],
                        ps_[0:DK, :], bc[:], MULT)

            pso_box = {}

            def p4_group(ot, c, scalar_copy=False):
                ssl = slice(c * CH, (c + 1) * CH)
                pso = pso_box["pso"][:, ot % 2, :]
                for p_ in range(NPAIR):
                    nc.tensor.matmul(
                        pso,
                        wo_sb[:, p_, ot * P:(ot + 1) * P],
                        att_sb[p_][:, ssl],
                        start=(p_ == 0), stop=(p_ == NPAIR - 1))
                ob = tmp.tile([P, CH], BF16, tag="ob")
                if scalar_copy:
                    nc.scalar.copy(out=ob[:], in_=pso)
                else:
                    nc.vector.tensor_copy(out=ob[:], in_=pso)
                nc.sync.dma_start(out[ot * P:(ot + 1) * P, ssl], ob[:])

            # prologue: projections for pair 0 chunk 0 only; the rest of
            # pair 0's P2 is hosted inside its own attention chunks
            st_cur = p2_prefetch(0)
            st_next = p2_prefetch(1)
            for fn in p2_hooks(st_cur, 0):
                fn()
            for pair in range(NPAIR):
                if pair == NPAIR - 1:
                    nc.sync.dma_start(wo_sb[:], wo3)
                    pso_box["pso"] = pp.tile([P, 2, CH], F32, tag="ps2", bufs=1, name="psot")
                for c in range(NCH):
                    hooks = {}
                    njt = 4 * c + 4

                    def add_unit(fns, lo, hi):
                        # spread 4 sub-hooks over jt slots [lo, hi)
                        span = max(1, hi - lo)
                        for idx, fn in enumerate(fns):
                            hooks.setdefault(lo + idx * span // 4, []).append(fn)

                    if pair == 0 and c < NCH - 1:
                        add_unit(p2_hooks(st_cur, c + 1), 0, njt // 2)
                        add_unit(p2_hooks(st_next, c), njt // 2, njt)
                    elif pair == 0:
                        add_unit(p2_hooks(st_next, c), 0, njt)
                    elif pair < NPAIR - 1:
                        add_unit(p2_hooks(st_next, c), njt // 5, njt)
                    elif c > 0:
                        # interleave O-projection of chunk c-1 into this chunk
                        npts = min(4, njt - 1)
                        for gi in range(8):
                            key = 1 + (gi % npts) * (njt - 1) // npts
                            hooks.setdefault(key, []).append(
                                lambda o=gi, cc=c - 1: p4_group(o, cc))
                    p3_chunk(pair, st_cur, c, hooks)
                st_cur = st_next
                st_next = p2_prefetch(pair + 2) if pair + 2 < NPAIR else None
            for ot in range(D // P):
                p4_group(ot, NCH - 1, scalar_copy=True)

            for c_ in reversed(pair_ctx):
                c_.__exit__(None, None, None)

    nc.compile()
    return nc


def _get_nc():
    global _CACHED_NC
    if _CACHED_NC is None:
        _CACHED_NC = build_nc()
    return _CACHED_NC


def make_in_maps(x, token_positions, Wq, Wk, Wv, Wo):
    import ml_dtypes
    bf = ml_dtypes.bfloat16
    f8 = ml_dtypes.float8_e4m3
    x = np.asarray(x, dtype=np.float32)
    pos = np.asarray(token_positions).astype(np.float64)

    freq_idx = np.arange(0, DK, 2, dtype=np.float64)
    inv_freq = 1.0 / (10000.0 ** (freq_idx / DK))
    ang = pos[:, None] * inv_freq[None, :]          # [S, DK/2]
    cos_t = np.cos(ang).astype(np.float32).T        # [DK/2, S]
    sin_t = np.sin(ang).astype(np.float32).T

    pidx = (np.arange(P) % DK) // 2
    cosn = np.ascontiguousarray(cos_t[pidx, :]).astype(bf)
    sgn = np.where(np.arange(P) % 2 == 0, -1.0, 1.0).astype(np.float32)
    sins = np.ascontiguousarray(sin_t[pidx, :] * sgn[:, None]).astype(bf)

    psw = np.zeros((P, P), dtype=np.float32)
    psw[np.arange(P), np.arange(P) ^ 1] = 1.0
    psw = psw.astype(bf)

    in_maps = []
    for core in range(8):
        b, g = core // 2, core % 2
        sl = slice(512 * g, 512 * g + 512)
        in_maps.append({
            "xT": np.ascontiguousarray(x[b].T).astype(bf),
            "x8": np.ascontiguousarray(x[b].T).astype(f8),
            "wq": (np.ascontiguousarray(np.asarray(Wq)[sl, :].T) * WSCALE).astype(f8),
            "wk": (np.ascontiguousarray(np.asarray(Wk)[sl, :].T) * WSCALE).astype(f8),
            "wv": np.ascontiguousarray(np.asarray(Wv)[sl, :].T).astype(bf),
            "wo": np.ascontiguousarray(np.asarray(Wo)[:, sl].T).astype(bf),
            "cosn": cosn,
            "sins": sins,
            "psw": psw,
        })
    return in_maps


def kernel(x, token_positions, Wq, Wk, Wv, Wo):
    global LAST_RESULTS
    nc = _get_nc()
    in_maps = make_in_maps(x, token_positions, Wq, Wk, Wv, Wo)
    res = run_bass_kernel_spmd(nc, in_maps, list(range(8)))
    LAST_RESULTS = res
    B = x.shape[0]
    outp = np.empty((B, S, D), dtype=np.float32)
    for b in range(B):
        outp[b] = (res.results[2 * b]["out"].astype(np.float32)
                   + res.results[2 * b + 1]["out"].astype(np.float32)).T
    return outp
